# revision 17
# baseline (speedup 1.0000x reference)
"""Trainium2 Bass kernel for the CNF-with-exact-Jacobian-trace problem.

Reference computation (B=2048, D=64, H=512):
    inp = [z, t]                      # time-augmented input, [D+1]
    h1  = tanh(inp @ W1 + b1)         # [H]
    h2  = tanh(h1 @ W2 + b2)          # [H]
    dz  = h2 @ W3 + b3                # [D]
    J   = d(dz)/dz                    # [D, D] per sample
    dlogp = -trace(J)

Key algebraic identity (avoids materializing J entirely):
    trace(J)_b = d1_b^T (W2 * (W3 @ W1z)^T) d2_b
where d1 = 1-h1^2, d2 = 1-h2^2, W1z = W1[:D], and * is elementwise.
So per sample the trace is a bilinear form through the H x H matrix
C = W2 * M^T with M = W3 @ W1z (computed on-device from the weights).

Distribution: pure data-parallel over B across 8 NeuronCores (256
samples each); weights replicated. Hidden-layer matmuls run
feature-on-partition ("transposed" activations) so biases are
per-partition ACT bias vectors; the final layer runs back in natural
layout so dz DMAs out contiguously.

Host-side work is layout-only (transpose/reshape/concat/replicate and
optional dtype cast of inputs) - no arithmetic happens on the host.
"""

import sys

sys.path.insert(0, "/opt/trn_rl_repo")

import numpy as np

import concourse.bacc as bacc
import concourse.mybir as mybir
from concourse import tile
from concourse.bass_utils import run_bass_kernel_spmd

B, D, H = 2048, 64, 512
NCORES = 8
BS = B // NCORES          # 256 samples per core
KA = D + 2                # 66: z rows + t row + ones row
HC = H // 128             # 4 chunks of the hidden dim
BC = BS // 128            # 2 sample chunks of 128

F32 = mybir.dt.float32
AF = mybir.ActivationFunctionType
ALU = mybir.AluOpType

# matmul operand dtype: "bf16" | "fp32" | "fp32r"
MM_DTYPE = "bf16"

_CACHED = {}
_RUN_KWARGS = {}  # test harness may set {"trace": True} for profiling


def build_nc(mm_dtype=None):
    mm_dtype = mm_dtype or MM_DTYPE
    MD = mybir.dt.bfloat16 if mm_dtype == "bf16" else F32
    use_r = mm_dtype == "fp32r"

    def mm_ap(ap):
        """AP passed to matmul: optionally reinterpret f32 as float32r."""
        return ap.bitcast(mybir.dt.float32r) if use_r else ap

    nc = bacc.Bacc("TRN2", target_bir_lowering=False, debug=False, num_devices=NCORES)

    # Packed inputs (fewer dma_starts -> less per-DMA fixed latency):
    #   pka [66, 768]   : cols 0:256 zaug | 256:768 w1b(=[W1;b1] rows)
    #   pkb [128, 2816] : cols 0:2048 w2 chunks | 2048:2304 w3 chunks
    #                     | 2304:2816 w3t (rows 0:64)
    #   pkc [128, 68]   : cols 0:4 b2 chunks | 4:68 b3 replicated
    pka = nc.declare_dram_parameter("pka", [KA, BS + H], MD, isOutput=False)
    pkb = nc.declare_dram_parameter("pkb", [128, 2816], MD, isOutput=False)
    pkc = nc.declare_dram_parameter("pkc", [128, 4 + D], F32, isOutput=False)
    dz_out = nc.declare_dram_parameter("dz", [BS, D], F32, isOutput=True)
    dlp_out = nc.declare_dram_parameter("dlp", [BS, 1], F32, isOutput=True)

    with tile.TileContext(nc) as tc:
        with (
            tc.tile_pool(name="w", bufs=1) as wp,
            tc.tile_pool(name="act", bufs=1) as ap,
            tc.tile_pool(name="psm", bufs=2, space="PSUM") as pm,
            tc.tile_pool(name="psa", bufs=3, space="PSUM") as pa,
            tc.tile_pool(name="pso", bufs=2, space="PSUM") as po,
            tc.tile_pool(name="pst", bufs=1, space="PSUM") as pt,
        ):
            # ---- loads: 3 packed DMAs on separate issue engines ----
            pka_t = wp.tile([KA, BS + H], MD)
            nc.sync.dma_start(pka_t[:], pka[:])
            pkb_t = wp.tile([128, 2816], MD)
            nc.scalar.dma_start(pkb_t[:], pkb[:])
            pkc_t = wp.tile([128, 4 + D], F32)
            nc.gpsimd.dma_start(pkc_t[:], pkc[:])

            zaug_t = pka_t[:, 0:BS]
            w1b_t = pka_t[:, BS : BS + H]

            def w2_blk(pc, qc):
                return pkb_t[:, pc * H + qc * 128 : pc * H + (qc + 1) * 128]

            def w2_row(pc):
                return pkb_t[:, pc * H : (pc + 1) * H]

            def w3_blk(qc):
                return pkb_t[:, 2048 + qc * D : 2048 + (qc + 1) * D]

            w3t_t = pkb_t[0:D, 2304 : 2304 + H]
            b2c_t = pkc_t[:, 0:4]
            b3r_t = pkc_t[:, 4 : 4 + D]

            ones_t = wp.tile([128, 1], MD)
            nc.vector.memset(ones_t[:], 1.0)

            # ---- PE warmup: dummy matmuls during the DMA-wait window keep
            # the PE busy so the HAM clock-gate reaches 8/8 before the real
            # matmul stream starts (otherwise the whole kernel runs at
            # 1.2 GHz) ----
            wu_t = wp.tile([128, H], MD)
            nc.vector.memset(wu_t[:], 1.0)
            for _ in range(6):
                ps_w = pm.tile([128, H], F32, tag="psm")
                nc.tensor.matmul(
                    ps_w[:], mm_ap(wu_t[:, 0:128]), mm_ap(wu_t[:]),
                    start=True, stop=True,
                )

            # ---- layer 1: A1T = [W1; W1t; b1]^T @ [zT; t; 1]  (K=66) ----
            h1 = ap.tile([128, HC, BS], MD)
            ps_a1 = []
            for hc in range(HC):
                ps = pa.tile([128, BS], F32, tag="psa")
                nc.tensor.matmul(
                    ps[:],
                    mm_ap(w1b_t[:, hc * 128 : (hc + 1) * 128]),
                    mm_ap(zaug_t[:]),
                    start=True,
                    stop=True,
                )
                ps_a1.append(ps)
            sq = ap.tile([128, HC, BS], F32)
            d1 = ap.tile([128, HC, BS], MD)
            for hc in range(HC):
                nc.scalar.activation(h1[:, hc, :], ps_a1[hc][:], AF.Tanh)
                nc.vector.tensor_mul(sq[:, hc, :], h1[:, hc, :], h1[:, hc, :])
                nc.vector.tensor_scalar(
                    d1[:, hc, :], sq[:, hc, :], -1.0, 1.0, ALU.mult, ALU.add
                )

            # ---- C = W2 * (W1z^T @ W3^T), chunked over p ----
            csb = wp.tile([128, HC, H], MD)
            for pc in range(HC):
                ps_m = pm.tile([128, H], F32, tag="psm")
                nc.tensor.matmul(
                    ps_m[:],
                    mm_ap(w1b_t[0:D, pc * 128 : (pc + 1) * 128]),
                    mm_ap(w3t_t[:]),
                    start=True,
                    stop=True,
                )
                nc.vector.tensor_mul(csb[:, pc, :], ps_m[:], w2_row(pc))

            # ---- layer 2: A2T = W2^T @ h1T + b2 ----
            h2 = ap.tile([128, HC, BS], MD)
            d2 = ap.tile([128, HC, BS], F32)
            for qc in range(HC):
                ps_a2 = pa.tile([128, BS], F32, tag="psa")
                for pc in range(HC):
                    nc.tensor.matmul(
                        ps_a2[:],
                        mm_ap(w2_blk(pc, qc)),
                        mm_ap(h1[:, pc, :]),
                        start=(pc == 0),
                        stop=(pc == HC - 1),
                    )
                nc.scalar.activation(
                    h2[:, qc, :], ps_a2[:], AF.Tanh, bias=b2c_t[:, qc : qc + 1]
                )
                nc.vector.tensor_mul(sq[:, qc, :], h2[:, qc, :], h2[:, qc, :])
                nc.vector.tensor_scalar(
                    d2[:, qc, :], sq[:, qc, :], -1.0, 1.0, ALU.mult, ALU.add
                )

            # ---- u = C^T @ d1, then E = u * d2 ----
            esb = ap.tile([128, HC, BS], MD)
            for qc in range(HC):
                ps_u = pa.tile([128, BS], F32, tag="psa")
                for pc in range(HC):
                    nc.tensor.matmul(
                        ps_u[:],
                        mm_ap(csb[:, pc, qc * 128 : (qc + 1) * 128]),
                        mm_ap(d1[:, pc, :]),
                        start=(pc == 0),
                        stop=(pc == HC - 1),
                    )
                nc.vector.tensor_mul(esb[:, qc, :], ps_u[:], d2[:, qc, :])

            # ---- layer 3 (natural layout): dz = h2 @ W3 + b3 ----
            # Emitted before the trace reduction: `out` only needs h2, so it
            # fills the PE while the DVE computes E = u * d2.
            dz_sb = ap.tile([128, BC, D], F32)
            for bc in range(BC):
                ps_o = po.tile([128, D], F32, tag="pso")
                for qc in range(HC):
                    nc.tensor.matmul(
                        ps_o[:],
                        mm_ap(h2[:, qc, bc * 128 : (bc + 1) * 128]),
                        mm_ap(w3_blk(qc)),
                        start=(qc == 0),
                        stop=(qc == HC - 1),
                    )
                nc.vector.tensor_add(dz_sb[:, bc, :], ps_o[:], b3r_t[:])
            nc.sync.dma_start(
                dz_out.rearrange("(n p) j -> p n j", p=128), dz_sb[:]
            )

            # ---- dlogp = -sum_q E[q, b]  (partition reduce via ones-matmul) ----
            ps_tr = pt.tile([1, BS], F32)
            for qc in range(HC):
                nc.tensor.matmul(
                    ps_tr[:],
                    mm_ap(ones_t[:]),
                    mm_ap(esb[:, qc, :]),
                    start=(qc == 0),
                    stop=(qc == HC - 1),
                )
            tr_sb = ap.tile([1, BS], F32)
            nc.scalar.mul(tr_sb[:], ps_tr[:], -1.0)
            nc.sync.dma_start(dlp_out.rearrange("b o -> o b"), tr_sb[:])

    nc.compile()
    return nc


def _np_md(mm_dtype):
    if mm_dtype == "bf16":
        import ml_dtypes

        return ml_dtypes.bfloat16
    return np.float32


def _prep_shared(t, W1, b1, W2, b2, W3, b3, mm_dtype):
    """Host-side layout prep of the replicated weight tensors (packed)."""
    md = _np_md(mm_dtype)
    # pkb [128, 2816]: w2 chunks | w3 chunks | w3t (rows 0:64)
    pkb = np.zeros((128, 2816), np.float32)
    pkb[:, 0:2048] = W2.reshape(HC, 128, H).transpose(1, 0, 2).reshape(128, 2048)
    pkb[:, 2048:2304] = W3.reshape(HC, 128, D).transpose(1, 0, 2).reshape(128, 256)
    pkb[0:D, 2304:2816] = W3.T
    # pkc [128, 68]: b2 chunks | b3 replicated
    pkc = np.zeros((128, 4 + D), np.float32)
    pkc[:, 0:4] = b2.reshape(HC, 128).T
    pkc[:, 4:] = b3[None, :]
    return dict(pkb=pkb.astype(md), pkc=pkc)


def kernel(z, logp_z, t, W1, b1, W2, b2, W3, b3):
    z = np.asarray(z, np.float32)
    t = np.asarray(t, np.float32)
    W1 = np.asarray(W1, np.float32)
    b1 = np.asarray(b1, np.float32)
    W2 = np.asarray(W2, np.float32)
    b2 = np.asarray(b2, np.float32)
    W3 = np.asarray(W3, np.float32)
    b3 = np.asarray(b3, np.float32)

    builder = _CACHED.get("builder", build_nc)
    key = ("nc", MM_DTYPE, builder.__name__)
    if key not in _CACHED:
        _CACHED[key] = builder(MM_DTYPE)
    nc = _CACHED[key]

    md = _np_md(MM_DTYPE)
    shared = _prep_shared(t, W1, b1, W2, b2, W3, b3, MM_DTYPE)
    w1b = np.concatenate([W1, b1[None, :]], axis=0)            # [66, 512]
    in_maps = []
    for c in range(NCORES):
        zs = z[c * BS : (c + 1) * BS]                          # [256, 64]
        pka = np.empty((KA, BS + H), np.float32)
        pka[:D, :BS] = zs.T
        pka[D, :BS] = t[0]
        pka[D + 1, :BS] = 1.0
        pka[:, BS:] = w1b
        in_maps.append({"pka": pka.astype(md), **shared})

    res = run_bass_kernel_spmd(nc, in_maps, list(range(NCORES)), **_RUN_KWARGS)
    _CACHED["last_results"] = res
    dz = np.concatenate([r["dz"] for r in res.results], axis=0)
    dlp = np.concatenate([r["dlp"] for r in res.results], axis=0)
    return dz, dlp


if __name__ == "__main__":
    rng = np.random.default_rng(0)
    inputs = {
        "z": rng.standard_normal((B, D)).astype(np.float32),
        "logp_z": np.zeros((B, 1), np.float32),
        "t": rng.random((1,)).astype(np.float32),
        "W1": (rng.standard_normal((D + 1, H)) / np.sqrt(D + 1)).astype(np.float32),
        "b1": np.zeros((H,), np.float32),
        "W2": (rng.standard_normal((H, H)) / np.sqrt(H)).astype(np.float32),
        "b2": np.zeros((H,), np.float32),
        "W3": (rng.standard_normal((H, D)) / np.sqrt(H)).astype(np.float32),
        "b3": np.zeros((D,), np.float32),
    }
    dz, dlp = kernel(**inputs)
    print(dz.shape, dlp.shape, dz.dtype, dlp.dtype)


# revision 18
# speedup vs baseline: 1.0633x; 1.0633x over previous
"""Trainium2 Bass kernel for the CNF-with-exact-Jacobian-trace problem.

Reference computation (B=2048, D=64, H=512):
    inp = [z, t]                      # time-augmented input, [D+1]
    h1  = tanh(inp @ W1 + b1)         # [H]
    h2  = tanh(h1 @ W2 + b2)          # [H]
    dz  = h2 @ W3 + b3                # [D]
    J   = d(dz)/dz                    # [D, D] per sample
    dlogp = -trace(J)

Key algebraic identity (avoids materializing J entirely):
    trace(J)_b = d1_b^T (W2 * (W3 @ W1z)^T) d2_b
where d1 = 1-h1^2, d2 = 1-h2^2, W1z = W1[:D], and * is elementwise.
So per sample the trace is a bilinear form through the H x H matrix
C = W2 * M^T with M = W3 @ W1z (computed on-device from the weights).

Distribution: pure data-parallel over B across 8 NeuronCores (256
samples each); weights replicated. Hidden-layer matmuls run
feature-on-partition ("transposed" activations) so biases are
per-partition ACT bias vectors; the final layer runs back in natural
layout so dz DMAs out contiguously.

Host-side work is layout-only (transpose/reshape/concat/replicate and
optional dtype cast of inputs) - no arithmetic happens on the host.
"""

import sys

sys.path.insert(0, "/opt/trn_rl_repo")

import numpy as np

import concourse.bacc as bacc
import concourse.mybir as mybir
from concourse import tile
from concourse.bass_utils import run_bass_kernel_spmd

B, D, H = 2048, 64, 512
NCORES = 8
BS = B // NCORES          # 256 samples per core
KA = D + 2                # 66: z rows + t row + ones row
HC = H // 128             # 4 chunks of the hidden dim
BC = BS // 128            # 2 sample chunks of 128

F32 = mybir.dt.float32
AF = mybir.ActivationFunctionType
ALU = mybir.AluOpType

# matmul operand dtype: "bf16" | "fp32" | "fp32r"
MM_DTYPE = "bf16"

_CACHED = {}
_RUN_KWARGS = {}  # test harness may set {"trace": True} for profiling


def build_nc(mm_dtype=None):
    mm_dtype = mm_dtype or MM_DTYPE
    MD = mybir.dt.bfloat16 if mm_dtype == "bf16" else F32
    use_r = mm_dtype == "fp32r"

    def mm_ap(ap):
        """AP passed to matmul: optionally reinterpret f32 as float32r."""
        return ap.bitcast(mybir.dt.float32r) if use_r else ap

    nc = bacc.Bacc("TRN2", target_bir_lowering=False, debug=False, num_devices=NCORES)

    zaug = nc.declare_dram_parameter("zaug", [KA, BS], MD, isOutput=False)
    w1b = nc.declare_dram_parameter("w1b", [KA, H], MD, isOutput=False)
    w2c = nc.declare_dram_parameter("w2c", [128, HC, H], MD, isOutput=False)
    w3c = nc.declare_dram_parameter("w3c", [128, HC, D], MD, isOutput=False)
    w3t = nc.declare_dram_parameter("w3t", [D, H], MD, isOutput=False)
    b2cc = nc.declare_dram_parameter("b2c", [128, HC], F32, isOutput=False)
    b3r = nc.declare_dram_parameter("b3r", [128, D], F32, isOutput=False)
    dz_out = nc.declare_dram_parameter("dz", [BS, D], F32, isOutput=True)
    dlp_out = nc.declare_dram_parameter("dlp", [BS, 1], F32, isOutput=True)

    with tile.TileContext(nc) as tc:
        with (
            tc.tile_pool(name="w", bufs=1) as wp,
            tc.tile_pool(name="act", bufs=1) as ap,
            tc.tile_pool(name="psm", bufs=2, space="PSUM") as pm,
            tc.tile_pool(name="psa", bufs=3, space="PSUM") as pa,
            tc.tile_pool(name="pso", bufs=2, space="PSUM") as po,
            tc.tile_pool(name="pst", bufs=1, space="PSUM") as pt,
        ):
            # ---- loads: w3t first (feeds the Mt matmuls that fill the
            # DMA-wait bubble), then A1 deps, spread across issue engines ----
            w3t_tt = wp.tile([D, H], MD)
            nc.sync.dma_start(w3t_tt[:], w3t[:])
            zaug_t = wp.tile([KA, BS], MD)
            nc.sync.dma_start(zaug_t[:], zaug[:])
            w1b_t = wp.tile([KA, H], MD)
            nc.scalar.dma_start(w1b_t[:], w1b[:])
            w2c_t = wp.tile([128, HC, H], MD)
            nc.scalar.dma_start(w2c_t[:], w2c[:])
            b2c_t = wp.tile([128, HC], F32)
            nc.gpsimd.dma_start(b2c_t[:], b2cc[:])
            b3r_t = wp.tile([128, D], F32)
            nc.gpsimd.dma_start(b3r_t[:], b3r[:])
            w3c_t = wp.tile([128, HC, D], MD)
            nc.gpsimd.dma_start(w3c_t[:], w3c[:])

            w3t_t = w3t_tt[:]

            def w2_blk(pc, qc):
                return w2c_t[:, pc, qc * 128 : (qc + 1) * 128]

            def w2_row(pc):
                return w2c_t[:, pc, :]

            def w3_blk(qc):
                return w3c_t[:, qc, :]

            ones_t = wp.tile([128, 1], MD)
            nc.vector.memset(ones_t[:], 1.0)

            # ---- C = W2 * (W1z^T @ W3^T): the Mt matmuls need only w3t +
            # w1b, so they run inside the DMA-wait bubble before A1 ----
            csb = wp.tile([128, HC, H], MD)
            for pc in range(HC):
                ps_m = pm.tile([128, H], F32, tag="psm")
                nc.tensor.matmul(
                    ps_m[:],
                    mm_ap(w1b_t[0:D, pc * 128 : (pc + 1) * 128]),
                    mm_ap(w3t_t[:]),
                    start=True,
                    stop=True,
                )
                nc.vector.tensor_mul(csb[:, pc, :], ps_m[:], w2_row(pc))

            # ---- layer 1: A1T = [W1; W1t; b1]^T @ [zT; t; 1]  (K=66) ----
            h1 = ap.tile([128, HC, BS], MD)
            ps_a1 = []
            for hc in range(HC):
                ps = pa.tile([128, BS], F32, tag="psa")
                nc.tensor.matmul(
                    ps[:],
                    mm_ap(w1b_t[:, hc * 128 : (hc + 1) * 128]),
                    mm_ap(zaug_t[:]),
                    start=True,
                    stop=True,
                )
                ps_a1.append(ps)
            sq = ap.tile([128, HC, BS], F32)
            d1 = ap.tile([128, HC, BS], MD)
            for hc in range(HC):
                nc.scalar.activation(h1[:, hc, :], ps_a1[hc][:], AF.Tanh)
                nc.vector.tensor_mul(sq[:, hc, :], h1[:, hc, :], h1[:, hc, :])
                nc.vector.tensor_scalar(
                    d1[:, hc, :], sq[:, hc, :], -1.0, 1.0, ALU.mult, ALU.add
                )

            # ---- layer 2: A2T = W2^T @ h1T + b2 ----
            h2 = ap.tile([128, HC, BS], MD)
            d2 = ap.tile([128, HC, BS], F32)
            for qc in range(HC):
                ps_a2 = pa.tile([128, BS], F32, tag="psa")
                for pc in range(HC):
                    nc.tensor.matmul(
                        ps_a2[:],
                        mm_ap(w2_blk(pc, qc)),
                        mm_ap(h1[:, pc, :]),
                        start=(pc == 0),
                        stop=(pc == HC - 1),
                    )
                nc.scalar.activation(
                    h2[:, qc, :], ps_a2[:], AF.Tanh, bias=b2c_t[:, qc : qc + 1]
                )
                nc.vector.tensor_mul(sq[:, qc, :], h2[:, qc, :], h2[:, qc, :])
                nc.vector.tensor_scalar(
                    d2[:, qc, :], sq[:, qc, :], -1.0, 1.0, ALU.mult, ALU.add
                )

            # ---- u = C^T @ d1, then E = u * d2 ----
            esb = ap.tile([128, HC, BS], MD)
            for qc in range(HC):
                ps_u = pa.tile([128, BS], F32, tag="psa")
                for pc in range(HC):
                    nc.tensor.matmul(
                        ps_u[:],
                        mm_ap(csb[:, pc, qc * 128 : (qc + 1) * 128]),
                        mm_ap(d1[:, pc, :]),
                        start=(pc == 0),
                        stop=(pc == HC - 1),
                    )
                nc.vector.tensor_mul(esb[:, qc, :], ps_u[:], d2[:, qc, :])

            # ---- layer 3 (natural layout): dz = h2 @ W3 + b3 ----
            # Emitted before the trace reduction: `out` only needs h2, so it
            # fills the PE while the DVE computes E = u * d2.
            dz_sb = ap.tile([128, BC, D], F32)
            for bc in range(BC):
                ps_o = po.tile([128, D], F32, tag="pso")
                for qc in range(HC):
                    nc.tensor.matmul(
                        ps_o[:],
                        mm_ap(h2[:, qc, bc * 128 : (bc + 1) * 128]),
                        mm_ap(w3_blk(qc)),
                        start=(qc == 0),
                        stop=(qc == HC - 1),
                    )
                nc.vector.tensor_add(dz_sb[:, bc, :], ps_o[:], b3r_t[:])
                nc.sync.dma_start(
                    dz_out.rearrange("(n p) j -> p n j", p=128)[:, bc, :],
                    dz_sb[:, bc, :],
                )

            # ---- dlogp = -sum_q E[q, b]  (partition reduce via ones-matmul) ----
            ps_tr = pt.tile([1, BS], F32)
            for qc in range(HC):
                nc.tensor.matmul(
                    ps_tr[:],
                    mm_ap(ones_t[:]),
                    mm_ap(esb[:, qc, :]),
                    start=(qc == 0),
                    stop=(qc == HC - 1),
                )
            tr_sb = ap.tile([1, BS], F32)
            nc.scalar.mul(tr_sb[:], ps_tr[:], -1.0)
            nc.scalar.dma_start(dlp_out.rearrange("b o -> o b"), tr_sb[:])

    nc.compile()
    return nc


def _np_md(mm_dtype):
    if mm_dtype == "bf16":
        import ml_dtypes

        return ml_dtypes.bfloat16
    return np.float32


def _prep_shared(t, W1, b1, W2, b2, W3, b3, mm_dtype):
    """Host-side layout prep of the replicated weight tensors."""
    md = _np_md(mm_dtype)
    w1b = np.concatenate([W1, b1[None, :]], axis=0).astype(md)      # [66, 512]
    w2c = np.ascontiguousarray(
        W2.reshape(HC, 128, H).transpose(1, 0, 2)).astype(md)
    w3c = np.ascontiguousarray(
        W3.reshape(HC, 128, D).transpose(1, 0, 2)).astype(md)
    w3t = np.ascontiguousarray(W3.T).astype(md)                     # [64, 512]
    b2c = np.ascontiguousarray(b2.reshape(HC, 128).T)               # [128, 4]
    b3r = np.ascontiguousarray(np.broadcast_to(b3, (128, D)))       # [128, 64]
    return dict(w1b=w1b, w2c=w2c, w3c=w3c, w3t=w3t, b2c=b2c, b3r=b3r)


def kernel(z, logp_z, t, W1, b1, W2, b2, W3, b3):
    z = np.asarray(z, np.float32)
    t = np.asarray(t, np.float32)
    W1 = np.asarray(W1, np.float32)
    b1 = np.asarray(b1, np.float32)
    W2 = np.asarray(W2, np.float32)
    b2 = np.asarray(b2, np.float32)
    W3 = np.asarray(W3, np.float32)
    b3 = np.asarray(b3, np.float32)

    builder = _CACHED.get("builder", build_nc)
    key = ("nc", MM_DTYPE, builder.__name__)
    if key not in _CACHED:
        _CACHED[key] = builder(MM_DTYPE)
    nc = _CACHED[key]

    md = _np_md(MM_DTYPE)
    shared = _prep_shared(t, W1, b1, W2, b2, W3, b3, MM_DTYPE)
    in_maps = []
    for c in range(NCORES):
        zs = z[c * BS : (c + 1) * BS]                          # [256, 64]
        zaug = np.empty((KA, BS), np.float32)
        zaug[:D] = zs.T
        zaug[D] = t[0]
        zaug[D + 1] = 1.0
        in_maps.append({"zaug": zaug.astype(md), **shared})

    res = run_bass_kernel_spmd(nc, in_maps, list(range(NCORES)), **_RUN_KWARGS)
    _CACHED["last_results"] = res
    dz = np.concatenate([r["dz"] for r in res.results], axis=0)
    dlp = np.concatenate([r["dlp"] for r in res.results], axis=0)
    return dz, dlp


if __name__ == "__main__":
    rng = np.random.default_rng(0)
    inputs = {
        "z": rng.standard_normal((B, D)).astype(np.float32),
        "logp_z": np.zeros((B, 1), np.float32),
        "t": rng.random((1,)).astype(np.float32),
        "W1": (rng.standard_normal((D + 1, H)) / np.sqrt(D + 1)).astype(np.float32),
        "b1": np.zeros((H,), np.float32),
        "W2": (rng.standard_normal((H, H)) / np.sqrt(H)).astype(np.float32),
        "b2": np.zeros((H,), np.float32),
        "W3": (rng.standard_normal((H, D)) / np.sqrt(H)).astype(np.float32),
        "b3": np.zeros((D,), np.float32),
    }
    dz, dlp = kernel(**inputs)
    print(dz.shape, dlp.shape, dz.dtype, dlp.dtype)
